# revision 22
# baseline (speedup 1.0000x reference)
"""Trainium2 Bass kernel for ConditionalLoRALinear.

Reference computation (f32):
    base = x @ W.T + b                      # [B,S,Do]
    lora = (x @ A.T) @ B.T * 2.0            # rank-8
    out  = base + lora * (ids == 7)         # per-token gate

Sharding over 8 NeuronCores: 2 token-halves x 4 d_out-quarters.
Each core holds its W-quarter (transposed, bf16, 8 MB) resident in
SBUF and streams its x-half (transposed, bf16) through in 128-token
strips.  All matmuls run in bf16 (full PE rate, same as float32r, but
half the HBM traffic and SBUF footprint) with f32 PSUM accumulation;
bf16 input rounding keeps the result well inside the 2e-2 relative
error budget.

Only ~1/64 of tokens are gated on (ids == COMP_TOKEN_ID), so the host
swaps the masked tokens into the first `nlora` (~2) strips of each
half (an involutive column swap touching only ~2*128 tokens); the
rank-8 LoRA path then runs on just those strips instead of all 64.
(128-wide matmuls pay a 4x PE penalty in f32r but run at full rate in
bf16.)  The per-token {0,2} gate is applied on DVE and the rank-8
update is added during the epilogue.  The output swap is undone on
host.
"""

import sys

for _p in ("/opt/trn_rl_repo",):
    if _p not in sys.path:
        sys.path.insert(0, _p)

from contextlib import ExitStack

import numpy as np

import concourse.bass as bass
import concourse.mybir as mybir
import concourse.tile as tile
from concourse import bacc
from concourse.bass import ts
from concourse.bass_utils import run_bass_kernel_spmd

F32 = mybir.dt.float32
BF16 = mybir.dt.bfloat16

B, S, DI, DO = 4, 4096, 4096, 4096
TOK = B * S              # 16384 tokens
NCORES = 8
TH = TOK // 2            # tokens per core (half)        = 8192
DQ = DO // 4             # d_out per core (quarter)      = 1024
P = 128                  # partition / strip size
KC = DI // P             # k-chunks                      = 32
NSTRIP = TH // P         # token strips per core         = 64
OC = DQ // 512           # 512-wide output chunks        = 2
COMP_TOKEN_ID = 7
SCALING = 2.0


def _build_nc(nlora):
    nc = bacc.Bacc(
        "TRN2",
        target_bir_lowering=False,
        debug=False,
        enable_asserts=True,
        num_devices=NCORES,
    )

    xT_d = nc.dram_tensor("xT", [NSTRIP, P, KC * P], BF16, kind="ExternalInput").ap()
    # W quarter in partition-major layout [P, KC*DQ] so each W-tile DMA
    # moves 8 KB contiguous per partition row (fast descriptors).
    wT_d = nc.dram_tensor("wT", [P, KC * DQ], BF16, kind="ExternalInput").ap()
    a8_d = nc.dram_tensor("a8", [P, KC * 8], BF16, kind="ExternalInput").ap()
    bT_d = nc.dram_tensor("bT", [8, DQ], BF16, kind="ExternalInput").ap()
    bias_d = nc.dram_tensor("biasr", [P, DQ], F32, kind="ExternalInput").ap()
    # per-token {0,2} gate for the lora strips, broadcast over 8 rank rows
    maskr_d = nc.dram_tensor("maskr", [8, nlora * P], F32, kind="ExternalInput").ap()
    out_d = nc.dram_tensor("out", [TH, DQ], F32, kind="ExternalOutput").ap()

    # pxa has only 2 PSUM slots, so the 3-strip prologue is only safe
    # when at most 2 of its strips carry the lora path.
    G = 3 if nlora <= 2 else 2

    with tile.TileContext(nc) as tc, ExitStack() as ctx:
        consts = ctx.enter_context(tc.tile_pool(name="consts", bufs=1))
        xpool = ctx.enter_context(tc.tile_pool(name="xp", bufs=G + 1))
        opool = ctx.enter_context(tc.tile_pool(name="op", bufs=3))
        spool = ctx.enter_context(tc.tile_pool(name="sp", bufs=1))
        psum = ctx.enter_context(tc.tile_pool(name="ps", bufs=G, space="PSUM"))

        # ---- small resident constants ahead of W ----
        at8 = consts.tile([P, KC * 8], BF16, name="at8", tag="at8")
        nc.sync.dma_start(at8[:], a8_d[:, :])
        bt = consts.tile([8, DQ], BF16, name="bt", tag="bt")
        nc.sync.dma_start(bt[:], bT_d[:, :])
        maskr8 = consts.tile([8, nlora * P], F32, name="maskr8", tag="maskr8")
        nc.sync.dma_start(maskr8[:], maskr_d[:, :])

        # The first G x strips load ahead of scalar's W share: first
        # halves of every prologue strip first (they gate the prologue
        # start), second halves interleaved with scalar's W tiles.
        hk = KC // 2
        xts0 = [
            xpool.tile([P, KC, P], BF16, name="xt", tag="xt") for _ in range(G)
        ]

        def xdma(s, half):
            nc.scalar.dma_start(
                xts0[s][:, ts(half, hk), :],
                xT_d[s, :, ts(half, hk * P)].rearrange("p (c t) -> p c t", t=P),
            )

        for s in range(G):
            xdma(s, 0)
        for s in range(G - 1):
            xdma(s, 1)

        # W: sync (HWDGE, otherwise idle) takes the early tiles; scalar
        # takes the late ones behind the prologue x halves.  gpsimd's
        # queue is slow to start (init drains), so it only gets
        # steady-state traffic.
        WB = 4  # K-chunks per W tile / DMA
        NW = KC // WB
        SP = NW - 3  # first SP tiles on sync, rest on scalar
        w_tiles = [
            consts.tile([P, WB, DQ], BF16, name=f"w{wb}", tag=f"w{wb}")
            for wb in range(NW)
        ]

        def wdma(wb, eng, split=1):
            # split early tiles into halves so chunk 0 lands sooner
            hb = WB // split
            for h in range(split):
                eng.dma_start(
                    w_tiles[wb][:, ts(h, hb), :],
                    wT_d[:, wb * WB * DQ + h * hb * DQ : wb * WB * DQ + (h + 1) * hb * DQ
                         ].rearrange("p (b o) -> p b o", o=DQ),
                )

        for wb in range(SP):
            wdma(wb, nc.sync, split=2 if wb < 2 else 1)
        wdma(SP, nc.scalar)
        xdma(G - 1, 1)
        for wb in range(SP + 1, NW):
            wdma(wb, nc.scalar)

        # bias (needed first at ~t+35us) loads behind W on sync
        biast = consts.tile([P, DQ], F32, name="biast", tag="biast")
        nc.sync.dma_start(biast[:], bias_d[:, :])

        def gate(st):
            """gate the rank-8 activations by the {0,2} token mask."""
            s, out_ps, xaT_ps = st
            if xaT_ps is None:
                return None
            xaT_g = spool.tile([8, P], BF16, name="xaT", tag="xaT", bufs=2)
            nc.vector.tensor_mul(xaT_g[:], xaT_ps[:], maskr8[:, ts(s, P)])
            return xaT_g

        def close_group(st, xaT_g):
            """rank-8 update accumulated straight into the open PSUM group."""
            s, out_ps, _ = st
            if xaT_g is not None:
                for j in range(OC):
                    nc.tensor.matmul(
                        out_ps[j][:], xaT_g[:], bt[:, ts(j, 512)],
                        start=False, stop=True,
                    )

        def epi_store(st):
            """bias on DVE, store per 512-wide half."""
            s, out_ps, _ = st
            ob = opool.tile([P, DQ], F32, name="ob", tag="ob")
            for j in range(OC):
                nc.vector.tensor_add(
                    ob[:, ts(j, 512)], out_ps[j][:], biast[:, ts(j, 512)]
                )
                # stores split across the two idle-at-tail queues so the
                # final strip's drain is shorter
                seng = nc.gpsimd if j == 0 else nc.scalar
                seng.dma_start(out_d[ts(s, P), ts(j, 512)], ob[:, ts(j, 512)])

        def base_mm(out_ps, xaT_ps, xt, c):
            lhsT = xt[:, c, :]
            # for lora strips the group stays open for the rank-8 update
            stop = (c == KC - 1) and xaT_ps is None
            for j in range(OC):
                nc.tensor.matmul(
                    out_ps[j][:],
                    lhsT,
                    w_tiles[c // WB][:, c % WB, ts(j, 512)],
                    start=(c == 0),
                    stop=stop,
                )
            if xaT_ps is not None:
                # rank-8 activations, transposed [r, tokens]; bf16
                # moving operand keeps this at full PE rate.
                nc.tensor.matmul(
                    xaT_ps[:],
                    at8[:, ts(c, 8)],
                    lhsT,
                    start=(c == 0),
                    stop=(c == KC - 1),
                )

        def new_psum(s):
            out_ps = [
                psum.tile([P, 512], F32, name=f"out_ps{j}", tag=f"po{j}")
                for j in range(OC)
            ]
            xaT_ps = (
                psum.tile([8, P], F32, name="xaT_ps", tag="pxa", bufs=2)
                if s < nlora
                else None
            )
            return (s, out_ps, xaT_ps)

        # ---- tile-major prologue: strips 0..G-1 interleaved per W tile,
        # so the PE consumes each W tile as it arrives instead of
        # stalling on the full 8 MB load.
        pro = [new_psum(s) for s in range(G)]
        for wb in range(NW):
            for s in range(G):
                for c in range(wb * WB, (wb + 1) * WB):
                    base_mm(pro[s][1], pro[s][2], xts0[s], c)
        # epilogues for all but the last prologue strip right away:
        # frees their PSUM slots for the steady strips
        for st in pro[:-1]:
            close_group(st, gate(st))
            epi_store(st)
        prev = pro[-1]

        epia_c = min(8, KC - 1)
        epib_c = min(16, KC - 1)
        for s in range(G, NSTRIP):
            xt = xpool.tile([P, KC, P], BF16, name="xt", tag="xt")
            xeng = nc.gpsimd if s % 2 == 0 else nc.scalar
            xeng.dma_start(xt[:], xT_d[s].rearrange("p (c t) -> p c t", t=P))
            st = new_psum(s)
            for c in range(KC):
                if c == epia_c and prev is not None:
                    close_group(prev, gate(prev))
                # previous strip's epilogue mid-stream: its PSUM slots are
                # released well before the next strip needs them, so the PE
                # never idles across a strip boundary.
                if c == epib_c and prev is not None:
                    epi_store(prev)
                    prev = None
                base_mm(st[1], st[2], xt, c)
            prev = st

        close_group(prev, gate(prev))
        epi_store(prev)

    nc.compile()
    return nc


_NC_CACHE = {}


def _get_nc(nlora):
    if nlora not in _NC_CACHE:
        _NC_CACHE[nlora] = _build_nc(nlora)
    return _NC_CACHE[nlora]


def _prep_host(x, ids):
    """Cast x to bf16, transpose, and swap masked tokens to the front of
    each half.

    Returns (xT [DI,TOK] bf16 with swapped columns, permuted {0,2} mask,
    per-half swap index pairs, nlora strip count)."""
    import ml_dtypes

    x2 = np.asarray(x, dtype=np.float32).reshape(TOK, DI).astype(ml_dtypes.bfloat16)
    xT = np.ascontiguousarray(x2.T)  # [DI, TOK]
    maskf = (np.asarray(ids).reshape(TOK) == COMP_TOKEN_ID).astype(
        np.float32
    ) * SCALING
    swaps = []
    counts = []
    for h in range(2):
        mh = maskf[h * TH : (h + 1) * TH]
        midx = np.nonzero(mh > 0)[0]
        k = len(midx)
        counts.append(k)
        need_move = midx[midx >= k]
        front_free = np.nonzero(mh[:k] == 0)[0]
        assert len(need_move) == len(front_free)
        swaps.append((front_free, need_move))
    nlora = max(1, max((k + P - 1) // P for k in counts))
    nlora = min(nlora, NSTRIP)

    maskp = maskf.copy()
    for h, (a, b) in enumerate(swaps):
        if len(a):
            ga = h * TH + a
            gb = h * TH + b
            tmp = xT[:, ga].copy()
            xT[:, ga] = xT[:, gb]
            xT[:, gb] = tmp
            mtmp = maskp[ga].copy()
            maskp[ga] = maskp[gb]
            maskp[gb] = mtmp
    return xT, maskp, swaps, nlora


def _make_in_maps(xT, maskp, nlora, W, b, lora_A, lora_B):
    import ml_dtypes

    WT = np.ascontiguousarray(
        np.asarray(W, dtype=np.float32).T.astype(ml_dtypes.bfloat16)
    )  # [DI, DO]
    BT = np.ascontiguousarray(np.asarray(lora_B, dtype=np.float32).T)  # [8, DO]
    AT = np.asarray(lora_A, dtype=np.float32).T  # [DI, 8]
    # [DI, 8] -> [P, KC*8] with a8[p, c*8+r] = A[r, c*128+p]
    a8_pre = np.ascontiguousarray(
        AT.reshape(KC, P, 8).transpose(1, 0, 2).reshape(P, KC * 8)
    ).astype(ml_dtypes.bfloat16)
    bias = np.asarray(b, dtype=np.float32)

    # strip-contiguous layout: xprep[s, p, c*128+t] = x[h*TH + s*128+t, c*128+p]
    xT_half = [
        np.ascontiguousarray(
            xT[:, h * TH : (h + 1) * TH]
            .reshape(KC, P, NSTRIP, P)
            .transpose(2, 1, 0, 3)
            .reshape(NSTRIP, P, KC * P)
        )
        for h in range(2)
    ]
    # {0,2} gate for the first nlora strips, broadcast over 8 rank rows
    mask_half = [
        np.ascontiguousarray(
            np.broadcast_to(
                maskp[h * TH : h * TH + nlora * P].reshape(1, nlora * P),
                (8, nlora * P),
            )
        )
        for h in range(2)
    ]
    # partition-major W layout: wprep[p, c*DQ+o] = W.T[c*128+p, o]
    wT_q = [
        np.ascontiguousarray(
            WT[:, q * DQ : (q + 1) * DQ]
            .reshape(KC, P, DQ)
            .transpose(1, 0, 2)
            .reshape(P, KC * DQ)
        )
        for q in range(4)
    ]
    bT_q = [
        np.ascontiguousarray(BT[:, q * DQ : (q + 1) * DQ]).astype(ml_dtypes.bfloat16)
        for q in range(4)
    ]
    bias_q = [
        np.ascontiguousarray(np.broadcast_to(bias[q * DQ : (q + 1) * DQ], (P, DQ)))
        for q in range(4)
    ]

    in_maps = []
    for c in range(NCORES):
        h, q = c // 4, c % 4
        in_maps.append(
            {
                "xT": xT_half[h],
                "wT": wT_q[q],
                "a8": a8_pre,
                "bT": bT_q[q],
                "biasr": bias_q[q],
                "maskr": mask_half[h],
            }
        )
    return in_maps


def kernel(x, ids, W, b, lora_A, lora_B):
    xT, maskp, swaps, nlora = _prep_host(x, ids)
    nc = _get_nc(nlora)
    in_maps = _make_in_maps(xT, maskp, nlora, W, b, lora_A, lora_B)
    results = run_bass_kernel_spmd(nc, in_maps, core_ids=list(range(NCORES)))
    out = np.empty((TOK, DO), dtype=np.float32)
    for c in range(NCORES):
        h, q = c // 4, c % 4
        out[h * TH : (h + 1) * TH, q * DQ : (q + 1) * DQ] = results.results[c]["out"]
    # undo the involutive token swap
    for h, (a, b_) in enumerate(swaps):
        if len(a):
            ga = h * TH + a
            gb = h * TH + b_
            tmp = out[ga].copy()
            out[ga] = out[gb]
            out[gb] = tmp
    return out.reshape(B, S, DO)


if __name__ == "__main__":
    rng = np.random.default_rng(0)
    x = rng.standard_normal((B, S, DI), dtype=np.float32)
    ids = rng.integers(0, 64, size=(B, S)).astype(np.int64)
    W = rng.standard_normal((DO, DI), dtype=np.float32) / np.sqrt(DI)
    b = (rng.standard_normal(DO) * 0.02).astype(np.float32)
    lora_A = rng.standard_normal((8, DI), dtype=np.float32) / np.sqrt(DI)
    lora_B = (rng.standard_normal((DO, 8)) * 0.02).astype(np.float32)
    out = kernel(x, ids, W, b, lora_A, lora_B)
    print(out.shape, out.dtype, float(np.abs(out).mean()))
